# revision 25
# baseline (speedup 1.0000x reference)
"""Trainium2 Bass kernel: DVS128-gesture spiking CNN inference (batch 8, T=4).

Sharding: data-parallel over batch N=8 -> 1 sample per NeuronCore, weights
replicated; the LIF membrane state lives in SBUF per core so the T-step scan
needs no cross-device traffic.

Per-core network (per timestep): 5x [3x3 SAME conv + BN + LIF + 2x2 maxpool]
then FC(1024->256)+LIF, FC(256->110)+LIF, grouped mean (110->11), accumulated
over T in PSUM.

v2 engine plan (vs v1: gpsimd 632us / vector 416us / scalar 205us / PE 170us):
 - Spike maps for conv inputs are +-0.5-encoded: s'' = s - 0.5, produced by a
   single DVE tensor_scalar (is_ge theta, subtract 0.5) on the POOLED membrane
   (max-then-threshold == threshold-then-max). Padding is -0.5; the constant
   -0.5 offset is folded into each conv's bias as +sum(W)/2 host-side, so the
   conv result is exact. No gpsimd thresholds, no scalar-table switches.
 - PSUM drains are plain contiguous f32->bf16 Copy on the scalar engine
   (natural column order; the v1 de-interleave made them 4x slower). At t=0
   the drain writes the membrane directly (v starts at 0: charge == drain).
 - 2x2 maxpool = TT max over grid-row pairs (contiguous streams, 2x mode)
   then TT max over column pairs (stride-2 reads), natural layout.
 - conv1 keeps the 2-taps-per-matmul pairing; its dup map sp1d[0:64]=spikes,
   sp1d[64:128]=spikes shifted one column left. Each pooled half is
   thresholded straight into its native partition range and two small
   SBUF->SBUF DMAs fill the opposite halves. conv1's bias rides a K=1 matmul
   against a constant ones row (sp1d has no ones row).
 - LIF hard reset v=(v<theta)*v stays a DVE STT, deferred off the critical
   path; resets are skipped entirely at t=T-1 (never consumed).
 - Membrane scaling: weights carry 2^(t-1) so charge is a plain add and
   thresholds compare against exact 2^t (v stored as w_t = 2^t * v_t).
"""

import numpy as np

C = 64
T = 4
NL = 5
N_CORES = 8
BN_EPS = 1e-5

RES = [128, 64, 32, 16, 8]  # conv layer input resolution
PAIRED = [True, True, True, False, True]
PF_L = [1024, 1024, 512, 256, 32]  # psum tile free size per layer

_BUILT = {}


def _build_nc(debug=False):
    import concourse.bass as bass  # noqa: F401
    import concourse.mybir as mybir
    import concourse.tile as tile
    from concourse import bacc
    from concourse.alu_op_type import AluOpType as Alu

    f32 = mybir.dt.float32
    mdt = mybir.dt.bfloat16

    nc = bacc.Bacc(None, target_bir_lowering=False)

    xim_d = nc.dram_tensor("xim", [T, 19, 32 * 4, 130], mdt, kind="ExternalInput")
    w0_d = nc.dram_tensor("conv0T", [19, T * 64], mdt, kind="ExternalInput")
    wl_d = nc.dram_tensor("convsT", [4, 65, T * 576], mdt, kind="ExternalInput")
    f1_d = nc.dram_tensor("fc1k", [128, T * 2048], mdt, kind="ExternalInput")
    f2_d = nc.dram_tensor("fc2k", [128, T * 220], mdt, kind="ExternalInput")
    c1p_d = nc.dram_tensor("conv1P", [128, T * 192], mdt, kind="ExternalInput")
    c1q_d = nc.dram_tensor("conv1Q", [128, T * 64], mdt, kind="ExternalInput")
    c2p_d = nc.dram_tensor("conv2P", [128, T * 192], mdt, kind="ExternalInput")
    c3p_d = nc.dram_tensor("conv3P", [128, T * 192], mdt, kind="ExternalInput")
    c1b_d = nc.dram_tensor("conv1B", [1, T * 64], mdt, kind="ExternalInput")
    bb_d = nc.dram_tensor("boostB", [110, 11], f32, kind="ExternalInput")
    out_d = nc.dram_tensor("out", [1, 11], f32, kind="ExternalOutput")

    with tile.TileContext(nc) as tc:
        with (
            tc.tile_pool(name="const", bufs=1) as cpool,
            tc.tile_pool(name="state", bufs=1) as spool,
            tc.tile_pool(name="bands", bufs=6) as bpool,
            tc.tile_pool(name="work", bufs=4) as wpool,
            tc.tile_pool(name="cpsum", bufs=3, space="PSUM") as ppool,
            tc.tile_pool(name="fcpsum", bufs=1, space="PSUM") as pfc,
            tc.tile_pool(name="accpsum", bufs=1, space="PSUM") as pacc,
        ):
            # ---- constants ----
            # conv0 weights first: the t=0 band DMAs queue right behind them,
            # so the PE can start within a few us. FC weights (2MB) load last.
            w0 = cpool.tile([19, T * 64], mdt)
            nc.sync.dma_start(w0[:, :], w0_d[:, :])
            c1p = cpool.tile([128, T * 192], mdt)
            c1b = cpool.tile([1, T * 64], mdt)
            wl = []
            for l in range(1, 5):
                wt = cpool.tile([65, T * 576], mdt, name=f"wl{l}", tag=f"wl{l}")
                wl.append(wt)
            f1 = cpool.tile([128, T * 2048], mdt)
            f2 = cpool.tile([128, T * 220], mdt)
            bb = cpool.tile([110, 11], f32)

            c1q = cpool.tile([128, T * 64], mdt)
            c2p = cpool.tile([128, T * 192], mdt)
            c3p = cpool.tile([128, T * 192], mdt)

            def emit_conv_weight_loads():
                nc.sync.dma_start(c1p[:, :], c1p_d[:, :])
                nc.sync.dma_start(c1b[:, :], c1b_d[:, :])
                nc.sync.dma_start(c1q[:, :], c1q_d[:, :])
                nc.sync.dma_start(c2p[:, :], c2p_d[:, :])
                nc.sync.dma_start(c3p[:, :], c3p_d[:, :])
                for l in range(1, 5):
                    nc.sync.dma_start(wl[l - 1][:, :], wl_d[l - 1])

            def emit_fc_weight_loads():
                nc.sync.dma_start(f1[:, :], f1_d[:, :])
                nc.sync.dma_start(f2[:, :], f2_d[:, :])
                nc.sync.dma_start(bb[:, :], bb_d[:, :])

            ones1 = cpool.tile([1, 512], mdt)
            nc.vector.memset(ones1[:, :], 1.0)

            # ---- state ----
            # membrane tiles (no memset needed: t=0 drain writes every element)
            v = []
            for l in range(5):
                pp = 128 if PAIRED[l] else 64
                g = RES[l] * RES[l] // 2 if PAIRED[l] else RES[l] * RES[l]
                vt = spool.tile([pp, g], mdt, name=f"v{l}", tag=f"v{l}")
                v.append(vt)
            vf1 = spool.tile([128, 2], f32)
            nc.vector.memset(vf1[:, :], 0.0)
            vf2 = spool.tile([110, 1], f32)
            nc.vector.memset(vf2[:, :], 0.0)

            # conv1 dup input map: [0:64]=padded spike map, [64:128]=same map
            # shifted one column left. Pads hold -0.5 (spike encoding).
            # Double-buffered by timestep parity so timestep t+1's pool writes
            # don't wait for conv1(t)'s reads.
            sp1ds = []
            for k in range(2):
                sp = spool.tile([128, 66, 66], mdt, name=f"sp1d{k}", tag=f"sp1d{k}")
                nc.vector.memset(sp[:, :, :], -0.5)
                sp1ds.append(sp)
            # row-shifted dup for conv1's (0,2)+(1,2) tap pair:
            # [0:64]=map, [64:128]=map shifted one row up.
            sp1e = spool.tile([128, 66, 66], mdt)
            nc.vector.memset(sp1e[:, :, :], -0.5)
            # col-shifted dups for conv2/conv3 tap pairs (dy,0)+(dy,1)
            spD = {2: spool.tile([128, 34, 34], mdt, name="spD2", tag="spD2"),
                  3: spool.tile([128, 18, 18], mdt, name="spD3", tag="spD3")}
            for sd in spD.values():
                nc.vector.memset(sd[:, :, :], -0.5)

            # spads for conv2..conv4: channels 0:64 (+-0.5 spikes, pads -0.5),
            # partition 64 = constant 1.0 bias row.
            spads = [None, None]
            for l in range(2, 5):
                hp = RES[l] + 2
                sp = spool.tile([65, hp, hp], mdt, name=f"spad{l}", tag=f"spad{l}")
                nc.vector.memset(sp[0:64, :, :], -0.5)
                nc.vector.memset(sp[64:65, :, :], 1.0)
                spads.append(sp)

            s4p = spool.tile([128, 8], mdt)   # {0,1} spikes feeding FC1
            s4m = spool.tile([128, 8], mdt)   # pooled membrane before threshold
            s1 = spool.tile([128, 2], mdt)
            s2 = spool.tile([110, 1], f32)

            acc_ps = pacc.tile([1, 11], f32)

            bands = {}
            deferred_resets = []

            def emit_bands(t):
                # band loads ride the gpsimd ring: the sync ring is reserved
                # for the latency-critical spike-map crossing/dup DMAs
                bs = []
                for bi in range(4):
                    bt = bpool.tile([19, 32, 130], mdt, name=f"band{bi}", tag="band")
                    nc.gpsimd.dma_start(bt[0:10, :, :], xim_d[t, 0:10, bi * 32 : (bi + 1) * 32, :])
                    nc.gpsimd.dma_start(bt[10:19, :, :], xim_d[t, 10:19, bi * 32 : (bi + 1) * 32, :])
                    bs.append(bt)
                bands[t] = bs

            def emit_layer(t, l):
                W = RES[l]
                paired = PAIRED[l]
                PF = PF_L[l]
                halfH = W // 2 if paired else W
                G = W * W // 2 if paired else W * W
                pp = 128 if paired else 64
                ntiles = G // PF
                rows_tile = PF // W
                banks = (PF + 511) // 512
                W2 = W // 2
                R2 = rows_tile // 2
                theta = float(2 ** t)
                sp1d = sp1ds[t % 2]

                for ti in range(ntiles):
                    pt = ppool.tile([pp, PF], f32, name="cps", tag="cps")
                    for b in range(banks):
                        cw = min(512, PF - b * 512)
                        rows_cw = cw // W
                        for half in range(2 if paired else 1):
                            r0 = (half * halfH if paired else 0) + ti * rows_tile + b * (512 // W)
                            oap = pt[64 * half : 64 * half + 64, b * 512 : b * 512 + cw]
                            if l == 0:
                                bi, r_loc = divmod(r0, 32)
                                nc.tensor.matmul(
                                    oap,
                                    w0[:, t * 64 : (t + 1) * 64],
                                    bands[t][bi][0:19, r_loc : r_loc + rows_cw, 0:128],
                                    start=True,
                                    stop=True,
                                )
                            elif l == 1:
                                d1r = sp1d
                                # bias via K=1 matmul against the ones row
                                nc.tensor.matmul(
                                    oap,
                                    c1b[0:1, t * 64 : (t + 1) * 64],
                                    ones1[0:1, 0:cw],
                                    start=True,
                                    stop=False,
                                )
                                for dy in range(3):
                                    # tap pair (dy,0)+(dy,1) via col-shift dup
                                    nc.tensor.matmul(
                                        oap,
                                        c1p[0:128, t * 192 + dy * 64 : t * 192 + dy * 64 + 64],
                                        d1r[0:128, r0 + dy : r0 + dy + rows_cw, 0:W],
                                        start=False,
                                        stop=False,
                                    )
                                # tap pair (0,2)+(1,2) via row-shift dup
                                nc.tensor.matmul(
                                    oap,
                                    c1q[0:128, t * 64 : (t + 1) * 64],
                                    sp1e[0:128, r0 : r0 + rows_cw, 2 : 2 + W],
                                    start=False,
                                    stop=False,
                                )
                                # remaining single tap (2,2)
                                nc.tensor.matmul(
                                    oap,
                                    wl[0][0:64, t * 576 + 8 * 64 : t * 576 + 8 * 64 + 64],
                                    d1r[0:64, r0 + 2 : r0 + 2 + rows_cw, 2 : 2 + W],
                                    start=False,
                                    stop=True,
                                )
                            elif l in (2, 3):
                                cXp = c2p if l == 2 else c3p
                                sd = spD[l]
                                for dy in range(3):
                                    # tap pair (dy,0)+(dy,1) via col-shift dup
                                    nc.tensor.matmul(
                                        oap,
                                        cXp[0:128, t * 192 + dy * 64 : t * 192 + dy * 64 + 64],
                                        sd[0:128, r0 + dy : r0 + dy + rows_cw, 0:W],
                                        start=(dy == 0),
                                        stop=False,
                                    )
                                for dy in range(3):
                                    # single tap (dy,2); last carries the bias row
                                    p = dy * 3 + 2
                                    kp = 65 if p == 8 else 64
                                    nc.tensor.matmul(
                                        oap,
                                        wl[l - 1][0:kp, t * 576 + p * 64 : t * 576 + p * 64 + 64],
                                        spads[l][0:kp, r0 + dy : r0 + dy + rows_cw, 2 : 2 + W],
                                        start=False,
                                        stop=(dy == 2),
                                    )
                            else:
                                for p in range(9):
                                    dy, dx = divmod(p, 3)
                                    kp = 65 if p == 8 else 64
                                    nc.tensor.matmul(
                                        oap,
                                        wl[l - 1][0:kp, t * 576 + p * 64 : t * 576 + p * 64 + 64],
                                        spads[l][0:kp, r0 + dy : r0 + dy + rows_cw, dx : dx + W],
                                        start=(p == 0),
                                        stop=(p == 8),
                                    )
                    vv = v[l][:, ti * PF : (ti + 1) * PF]
                    if t == 0:
                        # v starts at 0: the drain IS the charge. For L0 split
                        # chunks across scalar and DVE to halve the t=0 bubble.
                        if l == 0 and ti % 2 == 1:
                            nc.vector.tensor_copy(vv, pt[0:pp, 0:PF])
                        else:
                            nc.scalar.copy(vv, pt[0:pp, 0:PF])
                    else:
                        yb = wpool.tile([pp, PF], mdt, name="yb", tag="yb")
                        nc.scalar.copy(yb[0:pp, 0:PF], pt[0:pp, 0:PF])
                        nc.vector.tensor_tensor(vv, vv, yb[0:pp, 0:PF], Alu.add)
                    # pool stage 1: max over grid-row pairs (contiguous streams)
                    q1 = wpool.tile([pp, PF // 2], mdt, name="q1", tag="q1")
                    vra = v[l].rearrange("p (q two w) -> p q two w", two=2, w=W)
                    nc.vector.tensor_tensor(
                        q1[0:pp, :],
                        vra[0:pp, ti * R2 : (ti + 1) * R2, 0, :],
                        vra[0:pp, ti * R2 : (ti + 1) * R2, 1, :],
                        Alu.max,
                    )
                    # LIF hard reset, deferred (skipped at t=T-1: never consumed)
                    if t < T - 1:
                        deferred_resets.append((vv, theta))
                    # pool stage 2: max over column pairs -> pooled membrane
                    q1r = q1.rearrange("p (r w two) -> p r w two", r=R2, w=W2, two=2)
                    pr0 = ti * R2
                    if l == 0:
                        # native-partition writes into the dup map (membrane;
                        # thresholded in place below, halves crossed by DMA)
                        nc.vector.tensor_tensor(
                            sp1d[0:64, 1 + pr0 : 1 + pr0 + R2, 1:65],
                            q1r[0:64, :, :, 0], q1r[0:64, :, :, 1], Alu.max,
                        )
                        nc.vector.tensor_tensor(
                            sp1d[64:128, 33 + pr0 : 33 + pr0 + R2, 0:64],
                            q1r[64:128, :, :, 0], q1r[64:128, :, :, 1], Alu.max,
                        )
                    elif l < 4:
                        spn = spads[l + 1]
                        hb = halfH // 2
                        nc.vector.tensor_tensor(
                            spn[0:64, 1 + pr0 : 1 + pr0 + R2, 1 : 1 + W2],
                            q1r[0:64, :, :, 0], q1r[0:64, :, :, 1], Alu.max,
                        )
                        if paired:
                            nc.vector.tensor_tensor(
                                spn[0:64, 1 + hb + pr0 : 1 + hb + pr0 + R2, 1 : 1 + W2],
                                q1r[64:128, :, :, 0], q1r[64:128, :, :, 1], Alu.max,
                            )
                    else:
                        nc.vector.tensor_tensor(
                            s4m[0:128, :].rearrange("p (r w) -> p r w", r=R2, w=W2),
                            q1r[0:128, :, :, 0], q1r[0:128, :, :, 1], Alu.max,
                        )

                # per-layer threshold on the pooled membrane
                if l == 0:
                    nc.vector.tensor_scalar(
                        sp1d[0:64, 1:33, 1:65], sp1d[0:64, 1:33, 1:65],
                        theta, -0.5, Alu.is_ge, Alu.add,
                    )
                    nc.vector.tensor_scalar(
                        sp1d[64:128, 33:65, 0:64], sp1d[64:128, 33:65, 0:64],
                        theta, -0.5, Alu.is_ge, Alu.add,
                    )
                    # cross the halves (spikes) between partition ranges
                    nc.sync.dma_start(sp1d[0:64, 33:65, 1:65], sp1d[64:128, 33:65, 0:64])
                    nc.sync.dma_start(sp1d[64:128, 1:33, 0:64], sp1d[0:64, 1:33, 1:65])
                    # row-shift dup for the (0,2)+(1,2) tap pair, assembled
                    # straight from the two thresholded halves (parallel with
                    # the crossings, not chained behind them)
                    nc.scalar.dma_start(sp1e[0:64, 1:33, 0:66], sp1d[0:64, 1:33, 0:66])
                    nc.scalar.dma_start(sp1e[0:64, 33:65, 1:65], sp1d[64:128, 33:65, 0:64])
                    nc.gpsimd.dma_start(sp1e[64:128, 0:32, 0:66], sp1d[0:64, 1:33, 0:66])
                    nc.gpsimd.dma_start(sp1e[64:128, 32:64, 1:66], sp1d[64:128, 33:65, 0:65])
                elif l < 4:
                    spn = spads[l + 1]
                    HH = W2
                    nc.vector.tensor_scalar(
                        spn[0:64, 1 : 1 + HH, 1 : 1 + HH], spn[0:64, 1 : 1 + HH, 1 : 1 + HH],
                        theta, -0.5, Alu.is_ge, Alu.add,
                    )
                    if l + 1 in spD:
                        sd = spD[l + 1]
                        hp = HH + 2
                        nc.sync.dma_start(sd[0:64, 0:hp, 0:hp], spn[0:64, 0:hp, 0:hp])
                        nc.sync.dma_start(sd[64:128, 0:hp, 0 : hp - 1], spn[0:64, 0:hp, 1:hp])
                else:
                    nc.vector.tensor_scalar(
                        s4p[:, :], s4m[:, :], theta, None, Alu.is_ge
                    )

            def flush_resets():
                for vv, theta in deferred_resets:
                    nc.vector.scalar_tensor_tensor(
                        vv, vv, theta, vv, Alu.is_lt, Alu.mult
                    )
                deferred_resets.clear()

            def emit_fc(t):
                theta = float(2 ** t)
                ps1 = pfc.tile([128, 2], f32, name="fcps", tag="fcps")
                for h in range(2):
                    for f in range(8):
                        nc.tensor.matmul(
                            ps1[0:128, h : h + 1],
                            f1[:, t * 2048 + (f * 2 + h) * 128 : t * 2048 + (f * 2 + h + 1) * 128],
                            s4p[:, f : f + 1],
                            start=(f == 0),
                            stop=(f == 7),
                        )
                nc.vector.scalar_tensor_tensor(
                    vf1[:, :], vf1[:, :], 1.0, ps1[0:128, 0:2], Alu.mult, Alu.add
                )
                nc.vector.tensor_scalar(s1[:, :], vf1[:, :], theta, None, Alu.is_ge)
                if t < T - 1:
                    nc.vector.scalar_tensor_tensor(
                        vf1[:, :], vf1[:, :], theta, vf1[:, :], Alu.is_lt, Alu.mult
                    )

                ps2 = pfc.tile([110, 1], f32, name="fcps2", tag="fcps")
                for h in range(2):
                    nc.tensor.matmul(
                        ps2[0:110, 0:1],
                        f2[:, t * 220 + h * 110 : t * 220 + (h + 1) * 110],
                        s1[:, h : h + 1],
                        start=(h == 0),
                        stop=(h == 1),
                    )
                nc.vector.scalar_tensor_tensor(
                    vf2[:, :], vf2[:, :], 1.0, ps2[0:110, 0:1], Alu.mult, Alu.add
                )
                nc.vector.tensor_scalar(s2[:, :], vf2[:, :], theta, None, Alu.is_ge)
                if t < T - 1:
                    nc.vector.scalar_tensor_tensor(
                        vf2[:, :], vf2[:, :], theta, vf2[:, :], Alu.is_lt, Alu.mult
                    )
                nc.tensor.matmul(
                    acc_ps[0:1, 0:11],
                    s2[0:110, 0:1],
                    bb[0:110, 0:11],
                    start=(t == 0),
                    stop=(t == T - 1),
                )

            emit_bands(0)
            emit_layer(0, 0)
            emit_conv_weight_loads()
            emit_fc_weight_loads()
            emit_layer(0, 1)
            flush_resets()
            for t in range(T):
                if t + 1 < T:
                    emit_bands(t + 1)
                    emit_layer(t + 1, 0)
                emit_layer(t, 2)
                # conv1(t+1) sits between L2(t) and L3(t) in the PE queue so
                # the engine has work while the small layers' elementwise
                # dependency ladder (L2->L3->L4->FC) resolves.
                if t + 1 < T:
                    emit_layer(t + 1, 1)
                emit_layer(t, 3)
                emit_layer(t, 4)
                emit_fc(t)
                flush_resets()

            out_sb = spool.tile([1, 11], f32)
            nc.vector.tensor_copy(out_sb[:, :], acc_ps[0:1, 0:11])
            nc.sync.dma_start(out_d[0:1, 0:11], out_sb[:, :])

            if debug:
                dv0 = nc.dram_tensor("dbg_v0", [128, 8192], mdt, kind="ExternalOutput")
                nc.sync.dma_start(dv0[:, :], v[0][:, :])
                dv1 = nc.dram_tensor("dbg_v1", [128, 2048], mdt, kind="ExternalOutput")
                nc.sync.dma_start(dv1[:, :], v[1][:, :])
                dsp = nc.dram_tensor("dbg_sp1d", [128, 66 * 66], mdt, kind="ExternalOutput")
                nc.sync.dma_start(dsp[:, :], sp1d.rearrange("p a b -> p (a b)")[0:128])
                ds4 = nc.dram_tensor("dbg_s4p", [128, 8], mdt, kind="ExternalOutput")
                nc.sync.dma_start(ds4[:, :], s4p[:, :])

    nc.compile()
    return nc


def _prep_host(x, conv0_w, convs_w, bn_gamma, bn_beta, bn_mean, bn_var, fc1_w, fc2_w):
    f32 = np.float32
    x = np.asarray(x, f32)
    conv0_w = np.asarray(conv0_w, f32)
    convs_w = np.asarray(convs_w, f32)
    g = np.asarray(bn_gamma, f32) / np.sqrt(np.asarray(bn_var, f32) + BN_EPS)
    bconst = np.asarray(bn_beta, f32) - np.asarray(bn_mean, f32) * g
    fc1_w = np.asarray(fc1_w, f32)
    fc2_w = np.asarray(fc2_w, f32)

    n = x.shape[0]
    # per-timestep weight scale 2^(t-1): folds the LIF 1/tau=0.5 and lets the
    # membrane state be stored as w_t = 2^t * v_t (exact power-of-two scaling)
    ts_scale = np.array([2.0 ** (t - 1) for t in range(T)], f32)

    conv0T = np.zeros((19, T * 64), f32)
    convsT = np.zeros((4, 65, T * 576), f32)
    conv1P = np.zeros((128, T * 192), f32)
    conv1Q = np.zeros((128, T * 64), f32)
    conv2P = np.zeros((128, T * 192), f32)
    conv3P = np.zeros((128, T * 192), f32)
    conv1B = np.zeros((1, T * 64), f32)
    for t in range(T):
        sc = ts_scale[t]
        c0 = slice(t * 64, (t + 1) * 64)
        for p in range(9):
            dy, dx = divmod(p, 3)
            for ci in range(2):
                conv0T[2 * p + ci, c0] = sc * g[0] * conv0_w[:, ci, dy, dx]
        conv0T[18, c0] = sc * bconst[0]
        for l in range(1, 5):
            # +-0.5 spike encoding: full weights, bias += sum(W)/2 over all taps
            wsum = (g[l][:, None] * convs_w[l - 1].reshape(C, -1)).sum(1)
            bfull = bconst[l] + 0.5 * wsum
            for p in range(9):
                dy, dx = divmod(p, 3)
                convsT[l - 1, 0:64, t * 576 + p * 64 : t * 576 + (p + 1) * 64] = (
                    sc * g[l][None, :] * convs_w[l - 1][:, :, dy, dx].T
                )
            if l == 1:
                conv1B[0, c0] = sc * bfull
            else:
                convsT[l - 1, 64, t * 576 + 8 * 64 : t * 576 + 9 * 64] = sc * bfull
        for dy in range(3):
            for li, arr in ((1, conv1P), (2, conv2P), (3, conv3P)):
                arr[0:64, t * 192 + dy * 64 : t * 192 + (dy + 1) * 64] = (
                    sc * g[li][None, :] * convs_w[li - 1][:, :, dy, 0].T
                )
                arr[64:128, t * 192 + dy * 64 : t * 192 + (dy + 1) * 64] = (
                    sc * g[li][None, :] * convs_w[li - 1][:, :, dy, 1].T
                )
        # conv1 vertical pair (0,2)+(1,2)
        conv1Q[0:64, c0] = sc * g[1][None, :] * convs_w[0][:, :, 0, 2].T
        conv1Q[64:128, c0] = sc * g[1][None, :] * convs_w[0][:, :, 1, 2].T

    xpad = np.zeros((n, T, 2, 130, 130), f32)
    xpad[:, :, :, 1:129, 1:129] = x
    xim = np.zeros((n, T, 19, 130, 130), f32)
    for p in range(9):
        dy, dx = divmod(p, 3)
        for ci in range(2):
            xim[:, :, 2 * p + ci, 0:128, 0:128] = xpad[:, :, ci, dy : dy + 128, dx : dx + 128]
    xim[:, :, 18] = 1.0
    xim = np.ascontiguousarray(xim[:, :, :, 0:128, :])

    p_idx = np.arange(128)
    fc1k = np.zeros((128, T * 2048), f32)
    fc2k = np.zeros((128, T * 220), f32)
    for t in range(T):
        sc = ts_scale[t]
        for f in range(8):
            kcol = (p_idx % 64) * 16 + (p_idx // 64) * 8 + f
            for h in range(2):
                fc1k[:, t * 2048 + (f * 2 + h) * 128 : t * 2048 + (f * 2 + h + 1) * 128] = (
                    sc * fc1_w[h * 128 : (h + 1) * 128, kcol].T
                )
        for h in range(2):
            fc2k[:, t * 220 + h * 110 : t * 220 + (h + 1) * 110] = (
                sc * fc2_w[:, h * 128 : (h + 1) * 128].T
            )

    boostB = np.zeros((110, 11), f32)
    for k in range(110):
        boostB[k, k // 10] = 0.1

    import ml_dtypes

    bf16 = ml_dtypes.bfloat16
    xim, conv0T, convsT, conv1P, conv1Q, conv2P, conv3P, conv1B, fc1k, fc2k = (
        a.astype(bf16)
        for a in (xim, conv0T, convsT, conv1P, conv1Q, conv2P, conv3P, conv1B, fc1k, fc2k)
    )
    return xim, conv0T, convsT, conv1P, conv1Q, conv2P, conv3P, conv1B, fc1k, fc2k, boostB


def kernel(**inputs):
    import os

    from concourse.bass_utils import run_bass_kernel_spmd

    debug = bool(int(os.environ.get("KERNEL_DEBUG", "0")))

    x = np.asarray(inputs["x"], np.float32)
    assert x.shape == (8, 4, 2, 128, 128), x.shape
    xim, conv0T, convsT, conv1P, conv1Q, conv2P, conv3P, conv1B, fc1k, fc2k, boostB = _prep_host(
        x,
        inputs["conv0_w"],
        inputs["convs_w"],
        inputs["bn_gamma"],
        inputs["bn_beta"],
        inputs["bn_mean"],
        inputs["bn_var"],
        inputs["fc1_w"],
        inputs["fc2_w"],
    )

    if debug not in _BUILT:
        _BUILT[debug] = _build_nc(debug)
    nc = _BUILT[debug]

    shared = dict(conv0T=conv0T, convsT=convsT, conv1P=conv1P, conv1Q=conv1Q,
                  conv2P=conv2P, conv3P=conv3P, conv1B=conv1B,
                  fc1k=fc1k, fc2k=fc2k, boostB=boostB)
    in_maps = [dict(xim=np.ascontiguousarray(xim[n]), **shared) for n in range(N_CORES)]
    res = run_bass_kernel_spmd(nc, in_maps, core_ids=list(range(N_CORES)))
    global LAST_RESULT
    LAST_RESULT = res
    return np.stack([res.results[n]["out"][0] for n in range(N_CORES)], axis=0)


# revision 26
# speedup vs baseline: 1.1979x; 1.1979x over previous
"""Trainium2 Bass kernel: DVS128-gesture spiking CNN inference (batch 8, T=4).

Sharding: data-parallel over batch N=8 -> 1 sample per NeuronCore, weights
replicated; the LIF membrane state lives in SBUF per core so the T-step scan
needs no cross-device traffic.

Per-core network (per timestep): 5x [3x3 SAME conv + BN + LIF + 2x2 maxpool]
then FC(1024->256)+LIF, FC(256->110)+LIF, grouped mean (110->11), accumulated
over T in PSUM.

v2 engine plan (vs v1: gpsimd 632us / vector 416us / scalar 205us / PE 170us):
 - Spike maps for conv inputs are +-0.5-encoded: s'' = s - 0.5, produced by a
   single DVE tensor_scalar (is_ge theta, subtract 0.5) on the POOLED membrane
   (max-then-threshold == threshold-then-max). Padding is -0.5; the constant
   -0.5 offset is folded into each conv's bias as +sum(W)/2 host-side, so the
   conv result is exact. No gpsimd thresholds, no scalar-table switches.
 - PSUM drains are plain contiguous f32->bf16 Copy on the scalar engine
   (natural column order; the v1 de-interleave made them 4x slower). At t=0
   the drain writes the membrane directly (v starts at 0: charge == drain).
 - 2x2 maxpool = TT max over grid-row pairs (contiguous streams, 2x mode)
   then TT max over column pairs (stride-2 reads), natural layout.
 - conv1 keeps the 2-taps-per-matmul pairing; its dup map sp1d[0:64]=spikes,
   sp1d[64:128]=spikes shifted one column left. Each pooled half is
   thresholded straight into its native partition range and two small
   SBUF->SBUF DMAs fill the opposite halves. conv1's bias rides a K=1 matmul
   against a constant ones row (sp1d has no ones row).
 - LIF hard reset v=(v<theta)*v stays a DVE STT, deferred off the critical
   path; resets are skipped entirely at t=T-1 (never consumed).
 - Membrane scaling: weights carry 2^(t-1) so charge is a plain add and
   thresholds compare against exact 2^t (v stored as w_t = 2^t * v_t).
"""

import numpy as np

C = 64
T = 4
NL = 5
N_CORES = 8
BN_EPS = 1e-5

RES = [128, 64, 32, 16, 8]  # conv layer input resolution
PAIRED = [True, True, True, False, True]
PF_L = [1024, 1024, 512, 256, 32]  # psum tile free size per layer

_BUILT = {}


def _build_nc(debug=False):
    import concourse.bass as bass  # noqa: F401
    import concourse.mybir as mybir
    import concourse.tile as tile
    from concourse import bacc
    from concourse.alu_op_type import AluOpType as Alu

    f32 = mybir.dt.float32
    mdt = mybir.dt.bfloat16

    nc = bacc.Bacc(None, target_bir_lowering=False)

    xim_d = nc.dram_tensor("xim", [T, 19, 32 * 4, 130], mdt, kind="ExternalInput")
    w0_d = nc.dram_tensor("conv0T", [19, T * 64], mdt, kind="ExternalInput")
    wl_d = nc.dram_tensor("convsT", [4, 65, T * 576], mdt, kind="ExternalInput")
    f1_d = nc.dram_tensor("fc1k", [128, T * 2048], mdt, kind="ExternalInput")
    f2_d = nc.dram_tensor("fc2k", [128, T * 220], mdt, kind="ExternalInput")
    c1p_d = nc.dram_tensor("conv1P", [128, T * 192], mdt, kind="ExternalInput")
    c2p_d = nc.dram_tensor("conv2P", [128, T * 192], mdt, kind="ExternalInput")
    c3p_d = nc.dram_tensor("conv3P", [128, T * 192], mdt, kind="ExternalInput")
    c1b_d = nc.dram_tensor("conv1B", [1, T * 64], mdt, kind="ExternalInput")
    bb_d = nc.dram_tensor("boostB", [110, 11], f32, kind="ExternalInput")
    out_d = nc.dram_tensor("out", [1, 11], f32, kind="ExternalOutput")

    with tile.TileContext(nc) as tc:
        with (
            tc.tile_pool(name="const", bufs=1) as cpool,
            tc.tile_pool(name="state", bufs=1) as spool,
            tc.tile_pool(name="bands", bufs=6) as bpool,
            tc.tile_pool(name="work", bufs=4) as wpool,
            tc.tile_pool(name="cpsum", bufs=3, space="PSUM") as ppool,
            tc.tile_pool(name="fcpsum", bufs=1, space="PSUM") as pfc,
            tc.tile_pool(name="accpsum", bufs=1, space="PSUM") as pacc,
        ):
            # ---- constants ----
            # conv0 weights first: the t=0 band DMAs queue right behind them,
            # so the PE can start within a few us. FC weights (2MB) load last.
            w0 = cpool.tile([19, T * 64], mdt)
            nc.sync.dma_start(w0[:, :], w0_d[:, :])
            c1p = cpool.tile([128, T * 192], mdt)
            c1b = cpool.tile([1, T * 64], mdt)
            wl = []
            for l in range(1, 5):
                wt = cpool.tile([65, T * 576], mdt, name=f"wl{l}", tag=f"wl{l}")
                wl.append(wt)
            f1 = cpool.tile([128, T * 2048], mdt)
            f2 = cpool.tile([128, T * 220], mdt)
            bb = cpool.tile([110, 11], f32)

            c2p = cpool.tile([128, T * 192], mdt)
            c3p = cpool.tile([128, T * 192], mdt)

            def emit_conv_weight_loads():
                nc.sync.dma_start(c1p[:, :], c1p_d[:, :])
                nc.sync.dma_start(c1b[:, :], c1b_d[:, :])
                nc.sync.dma_start(c2p[:, :], c2p_d[:, :])
                nc.sync.dma_start(c3p[:, :], c3p_d[:, :])
                for l in range(1, 5):
                    nc.sync.dma_start(wl[l - 1][:, :], wl_d[l - 1])

            def emit_fc_weight_loads():
                nc.sync.dma_start(f1[:, :], f1_d[:, :])
                nc.sync.dma_start(f2[:, :], f2_d[:, :])
                nc.sync.dma_start(bb[:, :], bb_d[:, :])

            ones1 = cpool.tile([1, 512], mdt)
            nc.vector.memset(ones1[:, :], 1.0)

            # ---- state ----
            # membrane tiles (no memset needed: t=0 drain writes every element)
            v = []
            for l in range(5):
                pp = 128 if PAIRED[l] else 64
                g = RES[l] * RES[l] // 2 if PAIRED[l] else RES[l] * RES[l]
                vt = spool.tile([pp, g], mdt, name=f"v{l}", tag=f"v{l}")
                v.append(vt)
            vf1 = spool.tile([128, 2], f32)
            nc.vector.memset(vf1[:, :], 0.0)
            vf2 = spool.tile([110, 1], f32)
            nc.vector.memset(vf2[:, :], 0.0)

            # conv1 dup input map: [0:64]=padded spike map, [64:128]=same map
            # shifted one column left. Pads hold -0.5 (spike encoding).
            # Double-buffered by timestep parity so timestep t+1's pool writes
            # don't wait for conv1(t)'s reads.
            sp1ds = []
            for k in range(2):
                sp = spool.tile([128, 66, 66], mdt, name=f"sp1d{k}", tag=f"sp1d{k}")
                nc.vector.memset(sp[:, :, :], -0.5)
                sp1ds.append(sp)
            # row-shifted dups for conv2/conv3 tap pairs (0,dx)+(1,dx)
            spD = {2: spool.tile([128, 34, 34], mdt, name="spD2", tag="spD2"),
                  3: spool.tile([128, 18, 18], mdt, name="spD3", tag="spD3")}
            for sd in spD.values():
                nc.vector.memset(sd[:, :, :], -0.5)

            # spads for conv2..conv4: channels 0:64 (+-0.5 spikes, pads -0.5),
            # partition 64 = constant 1.0 bias row.
            spads = [None, None]
            for l in range(2, 5):
                hp = RES[l] + 2
                sp = spool.tile([65, hp, hp], mdt, name=f"spad{l}", tag=f"spad{l}")
                nc.vector.memset(sp[0:64, :, :], -0.5)
                nc.vector.memset(sp[64:65, :, :], 1.0)
                spads.append(sp)

            s4p = spool.tile([128, 8], mdt)   # {0,1} spikes feeding FC1
            s4m = spool.tile([128, 8], mdt)   # pooled membrane before threshold
            s1 = spool.tile([128, 2], mdt)
            s2 = spool.tile([110, 1], f32)

            acc_ps = pacc.tile([1, 11], f32)

            bands = {}
            deferred_resets = []

            def emit_bands(t):
                # band loads ride the gpsimd ring: the sync ring is reserved
                # for the latency-critical spike-map crossing/dup DMAs
                bs = []
                for bi in range(4):
                    bt = bpool.tile([19, 32, 130], mdt, name=f"band{bi}", tag="band")
                    nc.gpsimd.dma_start(bt[0:10, :, :], xim_d[t, 0:10, bi * 32 : (bi + 1) * 32, :])
                    nc.gpsimd.dma_start(bt[10:19, :, :], xim_d[t, 10:19, bi * 32 : (bi + 1) * 32, :])
                    bs.append(bt)
                bands[t] = bs

            def emit_layer(t, l):
                W = RES[l]
                paired = PAIRED[l]
                PF = PF_L[l]
                halfH = W // 2 if paired else W
                G = W * W // 2 if paired else W * W
                pp = 128 if paired else 64
                ntiles = G // PF
                rows_tile = PF // W
                banks = (PF + 511) // 512
                W2 = W // 2
                R2 = rows_tile // 2
                theta = float(2 ** t)
                sp1d = sp1ds[t % 2]

                for ti in range(ntiles):
                    pt = ppool.tile([pp, PF], f32, name="cps", tag="cps")
                    for b in range(banks):
                        cw = min(512, PF - b * 512)
                        rows_cw = cw // W
                        for half in range(2 if paired else 1):
                            r0 = (half * halfH if paired else 0) + ti * rows_tile + b * (512 // W)
                            oap = pt[64 * half : 64 * half + 64, b * 512 : b * 512 + cw]
                            if l == 0:
                                bi, r_loc = divmod(r0, 32)
                                nc.tensor.matmul(
                                    oap,
                                    w0[:, t * 64 : (t + 1) * 64],
                                    bands[t][bi][0:19, r_loc : r_loc + rows_cw, 0:128],
                                    start=True,
                                    stop=True,
                                )
                            elif l == 1:
                                d1r = sp1d
                                # bias via K=1 matmul against the ones row
                                nc.tensor.matmul(
                                    oap,
                                    c1b[0:1, t * 64 : (t + 1) * 64],
                                    ones1[0:1, 0:cw],
                                    start=True,
                                    stop=False,
                                )
                                for dx in range(3):
                                    # tap pair (0,dx)+(1,dx) via row-shift dup
                                    nc.tensor.matmul(
                                        oap,
                                        c1p[0:128, t * 192 + dx * 64 : t * 192 + dx * 64 + 64],
                                        d1r[0:128, r0 : r0 + rows_cw, dx : dx + W],
                                        start=False,
                                        stop=False,
                                    )
                                for dx in range(3):
                                    # single tap (2,dx)
                                    p = 6 + dx
                                    nc.tensor.matmul(
                                        oap,
                                        wl[0][0:64, t * 576 + p * 64 : t * 576 + p * 64 + 64],
                                        d1r[0:64, r0 + 2 : r0 + 2 + rows_cw, dx : dx + W],
                                        start=False,
                                        stop=(dx == 2),
                                    )
                            elif l in (2, 3):
                                cXp = c2p if l == 2 else c3p
                                sd = spD[l]
                                for dx in range(3):
                                    # tap pair (0,dx)+(1,dx) via row-shift dup
                                    nc.tensor.matmul(
                                        oap,
                                        cXp[0:128, t * 192 + dx * 64 : t * 192 + dx * 64 + 64],
                                        sd[0:128, r0 : r0 + rows_cw, dx : dx + W],
                                        start=(dx == 0),
                                        stop=False,
                                    )
                                for dx in range(3):
                                    # single tap (2,dx); last carries the bias row
                                    p = 6 + dx
                                    kp = 65 if p == 8 else 64
                                    nc.tensor.matmul(
                                        oap,
                                        wl[l - 1][0:kp, t * 576 + p * 64 : t * 576 + p * 64 + 64],
                                        spads[l][0:kp, r0 + 2 : r0 + 2 + rows_cw, dx : dx + W],
                                        start=False,
                                        stop=(dx == 2),
                                    )
                            else:
                                for p in range(9):
                                    dy, dx = divmod(p, 3)
                                    kp = 65 if p == 8 else 64
                                    nc.tensor.matmul(
                                        oap,
                                        wl[l - 1][0:kp, t * 576 + p * 64 : t * 576 + p * 64 + 64],
                                        spads[l][0:kp, r0 + dy : r0 + dy + rows_cw, dx : dx + W],
                                        start=(p == 0),
                                        stop=(p == 8),
                                    )
                    vv = v[l][:, ti * PF : (ti + 1) * PF]
                    if t == 0:
                        # v starts at 0: the drain IS the charge. For L0 split
                        # chunks across scalar and DVE to halve the t=0 bubble.
                        if l == 0 and ti % 2 == 1:
                            nc.vector.tensor_copy(vv, pt[0:pp, 0:PF])
                        else:
                            nc.scalar.copy(vv, pt[0:pp, 0:PF])
                    else:
                        yb = wpool.tile([pp, PF], mdt, name="yb", tag="yb")
                        nc.scalar.copy(yb[0:pp, 0:PF], pt[0:pp, 0:PF])
                        nc.vector.tensor_tensor(vv, vv, yb[0:pp, 0:PF], Alu.add)
                    # pool stage 1: max over grid-row pairs (contiguous streams)
                    q1 = wpool.tile([pp, PF // 2], mdt, name="q1", tag="q1")
                    vra = v[l].rearrange("p (q two w) -> p q two w", two=2, w=W)
                    nc.vector.tensor_tensor(
                        q1[0:pp, :],
                        vra[0:pp, ti * R2 : (ti + 1) * R2, 0, :],
                        vra[0:pp, ti * R2 : (ti + 1) * R2, 1, :],
                        Alu.max,
                    )
                    # LIF hard reset, deferred (skipped at t=T-1: never consumed)
                    if t < T - 1:
                        deferred_resets.append((vv, theta))
                    # pool stage 2: max over column pairs -> pooled membrane
                    q1r = q1.rearrange("p (r w two) -> p r w two", r=R2, w=W2, two=2)
                    pr0 = ti * R2
                    if l == 0:
                        # native-partition writes into the dup map (membrane;
                        # thresholded in place below, halves crossed by DMA)
                        nc.vector.tensor_tensor(
                            sp1d[0:64, 1 + pr0 : 1 + pr0 + R2, 1:65],
                            q1r[0:64, :, :, 0], q1r[0:64, :, :, 1], Alu.max,
                        )
                        nc.vector.tensor_tensor(
                            sp1d[64:128, 32 + pr0 : 32 + pr0 + R2, 1:65],
                            q1r[64:128, :, :, 0], q1r[64:128, :, :, 1], Alu.max,
                        )
                    elif l < 4:
                        spn = spads[l + 1]
                        hb = halfH // 2
                        nc.vector.tensor_tensor(
                            spn[0:64, 1 + pr0 : 1 + pr0 + R2, 1 : 1 + W2],
                            q1r[0:64, :, :, 0], q1r[0:64, :, :, 1], Alu.max,
                        )
                        if paired:
                            nc.vector.tensor_tensor(
                                spn[0:64, 1 + hb + pr0 : 1 + hb + pr0 + R2, 1 : 1 + W2],
                                q1r[64:128, :, :, 0], q1r[64:128, :, :, 1], Alu.max,
                            )
                    else:
                        nc.vector.tensor_tensor(
                            s4m[0:128, :].rearrange("p (r w) -> p r w", r=R2, w=W2),
                            q1r[0:128, :, :, 0], q1r[0:128, :, :, 1], Alu.max,
                        )

                # per-layer threshold on the pooled membrane
                if l == 0:
                    nc.vector.tensor_scalar(
                        sp1d[0:64, 1:33, 1:65], sp1d[0:64, 1:33, 1:65],
                        theta, -0.5, Alu.is_ge, Alu.add,
                    )
                    nc.vector.tensor_scalar(
                        sp1d[64:128, 32:64, 1:65], sp1d[64:128, 32:64, 1:65],
                        theta, -0.5, Alu.is_ge, Alu.add,
                    )
                    # cross the halves between partition ranges. Row-aligned
                    # (vertical dup), so both copies are one contiguous
                    # 32x66-element burst per partition.
                    nc.sync.dma_start(sp1d[0:64, 33:65, 0:66], sp1d[64:128, 32:64, 0:66])
                    nc.sync.dma_start(sp1d[64:128, 0:32, 0:66], sp1d[0:64, 1:33, 0:66])
                elif l < 4:
                    spn = spads[l + 1]
                    HH = W2
                    nc.vector.tensor_scalar(
                        spn[0:64, 1 : 1 + HH, 1 : 1 + HH], spn[0:64, 1 : 1 + HH, 1 : 1 + HH],
                        theta, -0.5, Alu.is_ge, Alu.add,
                    )
                    if l + 1 in spD:
                        sd = spD[l + 1]
                        hp = HH + 2
                        nc.sync.dma_start(sd[0:64, 0:hp, 0:hp], spn[0:64, 0:hp, 0:hp])
                        nc.sync.dma_start(sd[64:128, 0 : hp - 1, 0:hp], spn[0:64, 1:hp, 0:hp])
                else:
                    nc.vector.tensor_scalar(
                        s4p[:, :], s4m[:, :], theta, None, Alu.is_ge
                    )

            def flush_resets():
                for vv, theta in deferred_resets:
                    nc.vector.scalar_tensor_tensor(
                        vv, vv, theta, vv, Alu.is_lt, Alu.mult
                    )
                deferred_resets.clear()

            def emit_fc(t):
                theta = float(2 ** t)
                ps1 = pfc.tile([128, 2], f32, name="fcps", tag="fcps")
                for h in range(2):
                    for f in range(8):
                        nc.tensor.matmul(
                            ps1[0:128, h : h + 1],
                            f1[:, t * 2048 + (f * 2 + h) * 128 : t * 2048 + (f * 2 + h + 1) * 128],
                            s4p[:, f : f + 1],
                            start=(f == 0),
                            stop=(f == 7),
                        )
                nc.vector.scalar_tensor_tensor(
                    vf1[:, :], vf1[:, :], 1.0, ps1[0:128, 0:2], Alu.mult, Alu.add
                )
                nc.vector.tensor_scalar(s1[:, :], vf1[:, :], theta, None, Alu.is_ge)
                if t < T - 1:
                    nc.vector.scalar_tensor_tensor(
                        vf1[:, :], vf1[:, :], theta, vf1[:, :], Alu.is_lt, Alu.mult
                    )

                ps2 = pfc.tile([110, 1], f32, name="fcps2", tag="fcps")
                for h in range(2):
                    nc.tensor.matmul(
                        ps2[0:110, 0:1],
                        f2[:, t * 220 + h * 110 : t * 220 + (h + 1) * 110],
                        s1[:, h : h + 1],
                        start=(h == 0),
                        stop=(h == 1),
                    )
                nc.vector.scalar_tensor_tensor(
                    vf2[:, :], vf2[:, :], 1.0, ps2[0:110, 0:1], Alu.mult, Alu.add
                )
                nc.vector.tensor_scalar(s2[:, :], vf2[:, :], theta, None, Alu.is_ge)
                if t < T - 1:
                    nc.vector.scalar_tensor_tensor(
                        vf2[:, :], vf2[:, :], theta, vf2[:, :], Alu.is_lt, Alu.mult
                    )
                nc.tensor.matmul(
                    acc_ps[0:1, 0:11],
                    s2[0:110, 0:1],
                    bb[0:110, 0:11],
                    start=(t == 0),
                    stop=(t == T - 1),
                )

            emit_bands(0)
            emit_layer(0, 0)
            emit_conv_weight_loads()
            emit_fc_weight_loads()
            emit_layer(0, 1)
            flush_resets()
            for t in range(T):
                if t + 1 < T:
                    emit_bands(t + 1)
                    emit_layer(t + 1, 0)
                emit_layer(t, 2)
                # conv1(t+1) sits between L2(t) and L3(t) in the PE queue so
                # the engine has work while the small layers' elementwise
                # dependency ladder (L2->L3->L4->FC) resolves.
                if t + 1 < T:
                    emit_layer(t + 1, 1)
                emit_layer(t, 3)
                emit_layer(t, 4)
                emit_fc(t)
                flush_resets()

            out_sb = spool.tile([1, 11], f32)
            nc.vector.tensor_copy(out_sb[:, :], acc_ps[0:1, 0:11])
            nc.sync.dma_start(out_d[0:1, 0:11], out_sb[:, :])

            if debug:
                dv0 = nc.dram_tensor("dbg_v0", [128, 8192], mdt, kind="ExternalOutput")
                nc.sync.dma_start(dv0[:, :], v[0][:, :])
                dv1 = nc.dram_tensor("dbg_v1", [128, 2048], mdt, kind="ExternalOutput")
                nc.sync.dma_start(dv1[:, :], v[1][:, :])
                dsp = nc.dram_tensor("dbg_sp1d", [128, 66 * 66], mdt, kind="ExternalOutput")
                nc.sync.dma_start(dsp[:, :], sp1d.rearrange("p a b -> p (a b)")[0:128])
                ds4 = nc.dram_tensor("dbg_s4p", [128, 8], mdt, kind="ExternalOutput")
                nc.sync.dma_start(ds4[:, :], s4p[:, :])

    nc.compile()
    return nc


def _prep_host(x, conv0_w, convs_w, bn_gamma, bn_beta, bn_mean, bn_var, fc1_w, fc2_w):
    f32 = np.float32
    x = np.asarray(x, f32)
    conv0_w = np.asarray(conv0_w, f32)
    convs_w = np.asarray(convs_w, f32)
    g = np.asarray(bn_gamma, f32) / np.sqrt(np.asarray(bn_var, f32) + BN_EPS)
    bconst = np.asarray(bn_beta, f32) - np.asarray(bn_mean, f32) * g
    fc1_w = np.asarray(fc1_w, f32)
    fc2_w = np.asarray(fc2_w, f32)

    n = x.shape[0]
    # per-timestep weight scale 2^(t-1): folds the LIF 1/tau=0.5 and lets the
    # membrane state be stored as w_t = 2^t * v_t (exact power-of-two scaling)
    ts_scale = np.array([2.0 ** (t - 1) for t in range(T)], f32)

    conv0T = np.zeros((19, T * 64), f32)
    convsT = np.zeros((4, 65, T * 576), f32)
    conv1P = np.zeros((128, T * 192), f32)
    conv2P = np.zeros((128, T * 192), f32)
    conv3P = np.zeros((128, T * 192), f32)
    conv1B = np.zeros((1, T * 64), f32)
    for t in range(T):
        sc = ts_scale[t]
        c0 = slice(t * 64, (t + 1) * 64)
        for p in range(9):
            dy, dx = divmod(p, 3)
            for ci in range(2):
                conv0T[2 * p + ci, c0] = sc * g[0] * conv0_w[:, ci, dy, dx]
        conv0T[18, c0] = sc * bconst[0]
        for l in range(1, 5):
            # +-0.5 spike encoding: full weights, bias += sum(W)/2 over all taps
            wsum = (g[l][:, None] * convs_w[l - 1].reshape(C, -1)).sum(1)
            bfull = bconst[l] + 0.5 * wsum
            for p in range(9):
                dy, dx = divmod(p, 3)
                convsT[l - 1, 0:64, t * 576 + p * 64 : t * 576 + (p + 1) * 64] = (
                    sc * g[l][None, :] * convs_w[l - 1][:, :, dy, dx].T
                )
            if l == 1:
                conv1B[0, c0] = sc * bfull
            else:
                convsT[l - 1, 64, t * 576 + 8 * 64 : t * 576 + 9 * 64] = sc * bfull
        for dx in range(3):
            for li, arr in ((1, conv1P), (2, conv2P), (3, conv3P)):
                arr[0:64, t * 192 + dx * 64 : t * 192 + (dx + 1) * 64] = (
                    sc * g[li][None, :] * convs_w[li - 1][:, :, 0, dx].T
                )
                arr[64:128, t * 192 + dx * 64 : t * 192 + (dx + 1) * 64] = (
                    sc * g[li][None, :] * convs_w[li - 1][:, :, 1, dx].T
                )

    xpad = np.zeros((n, T, 2, 130, 130), f32)
    xpad[:, :, :, 1:129, 1:129] = x
    xim = np.zeros((n, T, 19, 130, 130), f32)
    for p in range(9):
        dy, dx = divmod(p, 3)
        for ci in range(2):
            xim[:, :, 2 * p + ci, 0:128, 0:128] = xpad[:, :, ci, dy : dy + 128, dx : dx + 128]
    xim[:, :, 18] = 1.0
    xim = np.ascontiguousarray(xim[:, :, :, 0:128, :])

    p_idx = np.arange(128)
    fc1k = np.zeros((128, T * 2048), f32)
    fc2k = np.zeros((128, T * 220), f32)
    for t in range(T):
        sc = ts_scale[t]
        for f in range(8):
            kcol = (p_idx % 64) * 16 + (p_idx // 64) * 8 + f
            for h in range(2):
                fc1k[:, t * 2048 + (f * 2 + h) * 128 : t * 2048 + (f * 2 + h + 1) * 128] = (
                    sc * fc1_w[h * 128 : (h + 1) * 128, kcol].T
                )
        for h in range(2):
            fc2k[:, t * 220 + h * 110 : t * 220 + (h + 1) * 110] = (
                sc * fc2_w[:, h * 128 : (h + 1) * 128].T
            )

    boostB = np.zeros((110, 11), f32)
    for k in range(110):
        boostB[k, k // 10] = 0.1

    import ml_dtypes

    bf16 = ml_dtypes.bfloat16
    xim, conv0T, convsT, conv1P, conv2P, conv3P, conv1B, fc1k, fc2k = (
        a.astype(bf16)
        for a in (xim, conv0T, convsT, conv1P, conv2P, conv3P, conv1B, fc1k, fc2k)
    )
    return xim, conv0T, convsT, conv1P, conv2P, conv3P, conv1B, fc1k, fc2k, boostB


def kernel(**inputs):
    import os

    from concourse.bass_utils import run_bass_kernel_spmd

    debug = bool(int(os.environ.get("KERNEL_DEBUG", "0")))

    x = np.asarray(inputs["x"], np.float32)
    assert x.shape == (8, 4, 2, 128, 128), x.shape
    xim, conv0T, convsT, conv1P, conv2P, conv3P, conv1B, fc1k, fc2k, boostB = _prep_host(
        x,
        inputs["conv0_w"],
        inputs["convs_w"],
        inputs["bn_gamma"],
        inputs["bn_beta"],
        inputs["bn_mean"],
        inputs["bn_var"],
        inputs["fc1_w"],
        inputs["fc2_w"],
    )

    if debug not in _BUILT:
        _BUILT[debug] = _build_nc(debug)
    nc = _BUILT[debug]

    shared = dict(conv0T=conv0T, convsT=convsT, conv1P=conv1P,
                  conv2P=conv2P, conv3P=conv3P, conv1B=conv1B,
                  fc1k=fc1k, fc2k=fc2k, boostB=boostB)
    in_maps = [dict(xim=np.ascontiguousarray(xim[n]), **shared) for n in range(N_CORES)]
    res = run_bass_kernel_spmd(nc, in_maps, core_ids=list(range(N_CORES)))
    global LAST_RESULT
    LAST_RESULT = res
    return np.stack([res.results[n]["out"][0] for n in range(N_CORES)], axis=0)
